# revision 7
# baseline (speedup 1.0000x reference)
"""Trainium2 Bass kernel for nn_Discriminator (embedding_lookup).

Computation per batch element b:
    ne = node_table[node_idx[b]]                  # [64]
    R  = relation_table[relation_idx[b]] as [64, 64]
    nb = node_table[node_neighbor_idx[b]]         # [64]
    out[b] = sigmoid( (ne @ R) . nb )

Strategy (8 NeuronCores, data-parallel over the batch):
  * The axon tunnel to the devices has ~50-80ms RTT per synchronous op but
    pipelines async work, so the per-call critical path is engineered to be
    one h2d (0.6MB of int32 indices) -> one fused XLA executable -> one d2h
    (0.29MB of scores), enqueued asynchronously with a single sync (~1 RTT).
  * The 25MB node table is replicated and kept DEVICE-RESIDENT across calls
    (cache keyed by object identity, then crc32 of the bytes) per the
    sharding hint; uploads go up sharded (25MB on the wire) and are
    replicated on-device by an all_gather over the chip interconnect. The
    block-diagonalized relation table is likewise device-resident.
  * The embedding gather + transposed `net` layout and the Bass kernel are
    FUSED into one XLA executable: the Bass program is built with
    target_bir_lowering=True so it embeds as an NKI custom_bir_kernel in
    the jit (a plain bass_exec NEFF costs an extra ~70ms round trip per
    execute on this tunnel; a fused NEFF pipelines like any XLA exec).
    Compiles are cached on disk (~3s reload in a fresh process).
  * Host: stable-sort batch by relation_idx, deal round-robin to 8 cores so
    each core's 8192 elements are relation-sorted; pad each of the 8 relation
    groups to a common capacity C (multiple of 128) -> 8*C slots = NT tiles
    of 128 elements (slot s -> partition s%128, tile s//128).
  * Device per core (raw bass, explicit semaphores):
      - stream net/nb span-chunks in via HWDGE DMAs (sync + scalar engines),
      - PE: one matmul per tile-pair: lhsT = net pair [128(2x64 d), 128(batch)],
        rhs = block-diagonal stacked relations -> temp [128, 128] in PSUM,
      - DVE: multiply+reduce temp x NB over 512-wide PSUM spans,
      - ACT: sigmoid, one DMA out of the [128, NT] score block.
  * Host: inverse-permute scores back to batch order.
"""
import sys, os, zlib

for _p in ("/opt/trn_rl_repo", "/root/.axon_site/_ro/trn_rl_repo"):
    if os.path.isdir(_p) and _p not in sys.path:
        sys.path.insert(0, _p)

import numpy as np
import concourse.bass as bass
import concourse.mybir as mybir
from concourse.bass_utils import run_bass_kernel_spmd

NODE_SIZE = 100000
D = 64
N_REL = 8
B = 65536
N_CORES = 8

_JAX_CACHE_DIR = "/var/tmp/jax_cache"

_PROGRAM_CACHE = {}


def build_program(NT, bir_lowering=False):
    """Per-core program. NT: number of 128-element tiles (multiple of 8)."""
    assert NT % 8 == 0
    NPAIR = NT // 2
    NSPAN = NT // 8      # 8 tiles (4 pairs, 512 temp columns) per span
    NCH = NSPAN          # one DMA chunk per span
    TPG = NT // N_REL    # tiles per relation group

    f32 = mybir.dt.float32

    nc = bass.Bass(target_bir_lowering=bir_lowering)
    # net[c, q, b]: partition c = par*64+d holds NE[d] of tile 2q+par, element b
    net_in = nc.dram_tensor("net", [128, NPAIR, 128], f32, kind="ExternalInput")
    nb_in = nc.dram_tensor("nbr", [128, NT, D], f32, kind="ExternalInput")
    # relcatz[:, g*128+0:64] = [R_g; 0], relcatz[:, g*128+64:128] = [0; R_g]
    relcatz = nc.dram_tensor("relcatz", [128, N_REL * 128], f32, kind="ExternalInput")
    out_sc = nc.dram_tensor("scores", [128, NT], f32, kind="ExternalOutput")

    # per-span matmul-instruction counts (pairs crossing a group boundary
    # need two half-width matmuls)
    def pair_tiles(q):
        return 2 * q, 2 * q + 1

    mm_per_span = [0] * NSPAN
    for q in range(NPAIR):
        tA, tB = pair_tiles(q)
        mm_per_span[tA // 8] += 1 if (tA // TPG == tB // TPG) else 2
    cum_mm = np.cumsum([0] + mm_per_span).tolist()

    from contextlib import ExitStack
    with ExitStack() as stack:
        ec = stack.enter_context
        s_relz = ec(nc.sbuf_tensor("sb_relz", [128, N_REL * 128], f32))
        s_net = ec(nc.sbuf_tensor("sb_net", [128, NPAIR, 128], f32))
        s_nb = ec(nc.sbuf_tensor("sb_nb", [128, NT, D], f32))
        s_prod = ec(nc.sbuf_tensor("sb_prod", [128, 8, D], f32))
        s_ssum = ec(nc.sbuf_tensor("sb_ssum", [128, NT], f32))
        s_scores = ec(nc.sbuf_tensor("sb_scores", [128, NT], f32))
        ps_tm = [ec(nc.psum_tensor(f"ps_tm{i}", [128, 512], f32)) for i in range(4)]
        s_ld = ec(nc.semaphore("s_ld"))
        s_mm = ec(nc.semaphore("s_mm"))
        s_dv = ec(nc.semaphore("s_dv"))
        s_pv = ec(nc.semaphore("s_pv"))
        s_sg = ec(nc.semaphore("s_sg"))
        s_out = ec(nc.semaphore("s_out"))
        block = ec(nc.Block())
        s_gc = [nc.alloc_semaphore(f"s_gc{c}") for c in range(NCH)]

        @block.sync
        def _(sync):
            # relz quartered across both HWDGE queues: shortens the head-of-line
            # delay ahead of the first net/nb chunks (-1.9us in the cost model)
            sync.dma_start(s_relz[:, 0:256], relcatz[:, 0:256]).then_inc(s_ld, 16)
            sync.dma_start(s_relz[:, 256:512], relcatz[:, 256:512]).then_inc(s_ld, 16)
            for c in range(NCH):
                sync.dma_start(
                    s_net[:, 4 * c: 4 * c + 4, :], net_in[:, 4 * c: 4 * c + 4, :]
                ).then_inc(s_gc[c], 16)
            sync.wait_ge(s_sg, NSPAN)
            sync.dma_start(out_sc[:], s_scores[:]).then_inc(s_out, 16)
            sync.wait_ge(s_out, 16)

        @block.scalar
        def _(scalar):
            scalar.dma_start(s_relz[:, 512:768], relcatz[:, 512:768]).then_inc(s_ld, 16)
            scalar.dma_start(s_relz[:, 768:1024], relcatz[:, 768:1024]).then_inc(s_ld, 16)
            for c in range(NCH):
                scalar.dma_start(
                    s_nb[:, 8 * c: 8 * c + 8, :], nb_in[:, 8 * c: 8 * c + 8, :]
                ).then_inc(s_gc[c], 16)
            for sp in range(NSPAN):
                scalar.wait_ge(s_dv, sp + 1)
                nc.scalar.activation(
                    s_scores[:, sp * 8: sp * 8 + 8],
                    s_ssum[:, sp * 8: sp * 8 + 8],
                    mybir.ActivationFunctionType.Sigmoid,
                ).then_inc(s_sg)

        @block.tensor
        def _(tensor):
            tensor.wait_ge(s_ld, 64)
            for sp in range(NSPAN):
                tensor.wait_ge(s_gc[sp], 32)
                if sp >= 4:
                    tensor.wait_ge(s_dv, sp - 3)  # WAR: temp bank reuse
                bank = ps_tm[sp % 4]
                cb = 0
                for q in range(4 * sp, 4 * sp + 4):
                    tA, tB = pair_tiles(q)
                    gA, gB = tA // TPG, tB // TPG
                    lhsT = s_net[:, q, :]
                    if gA == gB:
                        nc.tensor.matmul(
                            out=bank[:, cb + (tA % 8) * 64: cb + (tA % 8) * 64 + 128],
                            lhsT=lhsT,
                            rhs=s_relz[:, gA * 128: gA * 128 + 128],
                            start=True, stop=True,
                        ).then_inc(s_mm)
                    else:
                        nc.tensor.matmul(
                            out=bank[:, cb + (tA % 8) * 64: cb + (tA % 8) * 64 + 64],
                            lhsT=lhsT,
                            rhs=s_relz[:, gA * 128: gA * 128 + 64],
                            start=True, stop=True,
                        ).then_inc(s_mm)
                        nc.tensor.matmul(
                            out=bank[:, cb + (tB % 8) * 64: cb + (tB % 8) * 64 + 64],
                            lhsT=lhsT,
                            rhs=s_relz[:, gB * 128 + 64: gB * 128 + 128],
                            start=True, stop=True,
                        ).then_inc(s_mm)

        @block.vector
        def _(vector):
            for sp in range(NSPAN):
                vector.wait_ge(s_mm, cum_mm[sp + 1])
                vector.wait_ge(s_gc[sp], 32)       # NB chunk loaded
                if sp >= 1:
                    vector.wait_ge(s_dv, sp)       # WAR: prod reuse
                nc.vector.tensor_tensor(
                    out=s_prod[:, :, :],
                    in0=ps_tm[sp % 4][:].rearrange("p (a b) -> p a b", a=8),
                    in1=s_nb[:, sp * 8: sp * 8 + 8, :],
                    op=mybir.AluOpType.mult,
                ).then_inc(s_pv)
                vector.wait_ge(s_pv, sp + 1)
                nc.vector.tensor_reduce(
                    out=s_ssum[:, sp * 8: sp * 8 + 8],
                    in_=s_prod[:, :, :],
                    axis=mybir.AxisListType.X,
                    op=mybir.AluOpType.add,
                ).then_inc(s_dv)

    return nc


def _prep_host(node_idx, relation_idx, node_neighbor_idx):
    """Sort by relation, deal to cores, pad groups. Returns per-core int32
    index arrays [128, NT], posmap [N_CORES, 128, NT] (-1 = padding), NT."""
    node_idx = np.asarray(node_idx).astype(np.int32)
    relation_idx = np.asarray(relation_idx).astype(np.uint8)  # values < 8
    node_neighbor_idx = np.asarray(node_neighbor_idx).astype(np.int32)

    order = np.argsort(relation_idx, kind="stable")  # radix path on uint8
    core_pos = [order[k::N_CORES] for k in range(N_CORES)]
    counts = np.zeros((N_CORES, N_REL), np.int64)
    for k in range(N_CORES):
        counts[k] = np.bincount(relation_idx[core_pos[k]], minlength=N_REL)
    C = max(int(np.ceil(counts.max() / 128.0) * 128), 128)
    NT = (N_REL * C) // 128

    ne = np.zeros((N_CORES, 128, NT), np.int32)
    nb = np.zeros((N_CORES, 128, NT), np.int32)
    posmap = np.full((N_CORES, 128, NT), -1, np.int64)
    for k in range(N_CORES):
        pos = core_pos[k]
        cnt = counts[k]
        starts = np.repeat(np.arange(N_REL) * C, cnt)
        group_base = np.repeat(np.cumsum(cnt) - cnt, cnt)
        within = np.arange(len(pos)) - group_base
        s = starts + within
        t, p = s // 128, s % 128
        ne[k, p, t] = node_idx[pos]
        nb[k, p, t] = node_neighbor_idx[pos]
        posmap[k, p, t] = pos
    return ne, nb, posmap, NT


def _build_relcatz(relation_table):
    rt = np.asarray(relation_table, np.float32).reshape(N_REL, D, D)
    relz = np.zeros((128, N_REL * 128), np.float32)
    for g in range(N_REL):
        relz[0:64, g * 128: g * 128 + 64] = rt[g]
        relz[64:128, g * 128 + 64: g * 128 + 128] = rt[g]
    return relz


# ---------------------------------------------------------------------------
# jax plumbing: mesh, persistent compile cache, device-resident tables
# ---------------------------------------------------------------------------

_MESH = None
_TABLE_CACHE = {}    # "node"/"relz" -> (crc_key, device_array, host_ref)
_REPL_FN = {}        # n_rows -> jitted all_gather replicator
_FUSED_CACHE = {}    # NT -> fused fast-dispatch Compiled
_RUNNER_CACHE = {}   # NT -> non-lowered bass runner (fallback)
_GATHER_CACHE = {}   # NT -> gather-only jit (fallback)
_SCORES_POOL = {}    # NT -> device array donated as next out-buffer (fallback)


def _init_jax():
    import jax
    try:
        if not jax.config.jax_compilation_cache_dir:
            jax.config.update("jax_compilation_cache_dir", _JAX_CACHE_DIR)
            jax.config.update("jax_persistent_cache_min_compile_time_secs", 0.0)
            jax.config.update("jax_persistent_cache_min_entry_size_bytes", 0)
    except Exception:
        pass
    return jax


def _get_mesh():
    global _MESH
    if _MESH is None:
        jax = _init_jax()
        from concourse import bass2jax
        _MESH = bass2jax.Mesh(np.asarray(jax.devices()[:N_CORES]), ("core",))
    return _MESH


def _crc(a):
    a = np.ascontiguousarray(a)
    return (a.shape, a.dtype.str, zlib.crc32(memoryview(a.reshape(-1))))


def _replicate(arr):
    """Device-replicated copy of a host array: upload sharded (1x bytes on
    the slow tunnel) and all_gather on-device; plain device_put fallback."""
    jax = _init_jax()
    from jax.sharding import NamedSharding, PartitionSpec as P
    mesh = _get_mesh()
    n = arr.shape[0]
    if n % N_CORES == 0:
        try:
            if n not in _REPL_FN:
                from jax.experimental.shard_map import shard_map

                def body(shard):
                    return jax.lax.all_gather(shard, "core", axis=0, tiled=True)

                _REPL_FN[n] = jax.jit(shard_map(
                    body, mesh=mesh, in_specs=(P("core"),), out_specs=P(),
                    check_rep=False))
            shard = jax.device_put(arr, NamedSharding(mesh, P("core")))
            dev = _REPL_FN[n](shard)
            jax.block_until_ready(dev)
            return dev
        except Exception:
            pass
    dev = jax.device_put(arr, NamedSharding(mesh, P()))
    jax.block_until_ready(dev)
    return dev


def _dev_table(node_table):
    """Replicated device-resident node table, cached by identity then crc."""
    hit = _TABLE_CACHE.get("node")
    if hit is not None and hit[2] is node_table:
        return hit[1]
    key = _crc(node_table)
    if hit is not None and hit[0] == key:
        _TABLE_CACHE["node"] = (key, hit[1], node_table)
        return hit[1]
    dev = _replicate(node_table)
    _TABLE_CACHE["node"] = (key, dev, node_table)
    return dev


def _dev_relz(relation_table):
    """P('core')-sharded [8*128, 1024] block-diagonal relation table."""
    jax = _init_jax()
    from jax.sharding import NamedSharding, PartitionSpec as P
    hit = _TABLE_CACHE.get("relz")
    if hit is not None and hit[2] is relation_table:
        return hit[1]
    key = _crc(np.asarray(relation_table, np.float32))
    if hit is not None and hit[0] == key:
        _TABLE_CACHE["relz"] = (key, hit[1], relation_table)
        return hit[1]
    relz = _build_relcatz(relation_table)
    tiled = np.tile(relz, (N_CORES, 1))
    dev = jax.device_put(tiled, NamedSharding(_get_mesh(), P("core")))
    jax.block_until_ready(dev)
    _TABLE_CACHE["relz"] = (key, dev, relation_table)
    return dev


# ---------------------------------------------------------------------------
# Primary path: gather + bass kernel fused into ONE XLA executable via the
# NKI/BIR-lowering pipeline, AOT-compiled with the bass effect suppressed.
# ---------------------------------------------------------------------------

def _get_fused(NT):
    if NT in _FUSED_CACHE:
        return _FUSED_CACHE[NT]
    jax = _init_jax()
    from concourse import bass2jax
    from jax.sharding import NamedSharding, PartitionSpec as P
    bass2jax.install_neuronx_cc_hook()
    mesh = _get_mesh()
    shc = NamedSharding(mesh, P("core"))

    nc = build_program(NT, bir_lowering=True)
    if not nc.is_finalized():
        nc.finalize()
    in_names, out_names, out_avals = [], [], []
    pn = nc.partition_id_tensor.name if nc.partition_id_tensor else None
    for alloc in nc.m.functions[0].allocations:
        if not isinstance(alloc, mybir.MemoryLocationSet):
            continue
        name = alloc.memorylocations[0].name
        if alloc.kind == "ExternalInput":
            if name != pn:
                in_names.append(name)
        elif alloc.kind == "ExternalOutput":
            out_names.append(name)
            out_avals.append(jax.core.ShapedArray(
                tuple(alloc.tensor_shape), mybir.dt.np(alloc.dtype)))
    assert in_names == ["net", "nbr", "relcatz"] and out_names == ["scores"]
    all_names = list(in_names) + ([pn] if pn else [])

    def fused_body(tbl, ix, relz):
        g = tbl[ix[:, 0, :]]                       # [128, NT, 64]
        net = g.reshape(128, NT // 2, 2, D).transpose(2, 3, 1, 0)
        net = net.reshape(128, NT // 2, 128)
        nbr = tbl[ix[:, 1, :]]                     # [128, NT, 64]
        operands = [net, nbr, relz]
        if pn:
            operands.append(bass2jax.partition_id_tensor())
        outs = bass2jax._bass_exec_p.bind(
            *operands, out_avals=tuple(out_avals), in_names=tuple(all_names),
            out_names=tuple(out_names), lowering_input_output_aliases=(),
            sim_require_finite=True, sim_require_nnan=True, nc=nc)
        return outs[0]

    arg_specs = [
        jax.ShapeDtypeStruct((NODE_SIZE, D), np.float32,
                             sharding=NamedSharding(mesh, P())),
        jax.ShapeDtypeStruct((N_CORES * 128, 2, NT), np.int32, sharding=shc),
        jax.ShapeDtypeStruct((N_CORES * 128, N_REL * 128), np.float32,
                             sharding=shc),
    ]

    def compile_fn():
        jfn = jax.jit(bass2jax.shard_map(
            fused_body, mesh=mesh, in_specs=(P(), P("core"), P("core")),
            out_specs=P("core"), check_rep=False))
        return jfn.lower(*arg_specs).compile()

    fn = bass2jax.fast_dispatch_compile(compile_fn)
    _FUSED_CACHE[NT] = fn
    return fn


def _run_fused(NT, table_dev, relz_dev, ne, nb):
    jax = _init_jax()
    from jax.sharding import NamedSharding, PartitionSpec as P
    fn = _get_fused(NT)
    ix = np.stack([ne, nb], axis=2).reshape(N_CORES * 128, 2, NT)
    ix_dev = jax.device_put(ix, NamedSharding(_get_mesh(), P("core")))
    return np.asarray(fn(table_dev, ix_dev, relz_dev))


# ---------------------------------------------------------------------------
# Fallback 1: separate gather jit + non-lowered bass NEFF (device-resident)
# ---------------------------------------------------------------------------

def _get_runner(nc, NT):
    """Cached jitted executor for the non-lowered program."""
    if NT in _RUNNER_CACHE:
        return _RUNNER_CACHE[NT]
    jax = _init_jax()
    from concourse import bass2jax
    bass2jax.install_neuronx_cc_hook()
    in_names, out_names, out_avals, out_shapes, in_shapes = [], [], [], [], []
    partition_name = nc.partition_id_tensor.name if nc.partition_id_tensor else None
    for alloc in nc.m.functions[0].allocations:
        if not isinstance(alloc, mybir.MemoryLocationSet):
            continue
        name = alloc.memorylocations[0].name
        if alloc.kind == "ExternalInput":
            if name != partition_name:
                in_names.append(name)
                in_shapes.append((tuple(alloc.tensor_shape), mybir.dt.np(alloc.dtype)))
        elif alloc.kind == "ExternalOutput":
            shape = tuple(alloc.tensor_shape)
            dtype = mybir.dt.np(alloc.dtype)
            out_names.append(name)
            out_avals.append(jax.core.ShapedArray(shape, dtype))
            out_shapes.append((shape, dtype))
    n_params = len(in_names)
    all_names = list(in_names) + list(out_names)
    if partition_name is not None:
        all_names.append(partition_name)

    def _body(*args):
        operands = list(args)
        if partition_name is not None:
            operands.append(bass2jax.partition_id_tensor())
        outs = bass2jax._bass_exec_p.bind(
            *operands, out_avals=tuple(out_avals), in_names=tuple(all_names),
            out_names=tuple(out_names), lowering_input_output_aliases=(),
            sim_require_finite=True, sim_require_nnan=True, nc=nc)
        return tuple(outs)

    mesh = _get_mesh()
    in_specs = (bass2jax.PartitionSpec("core"),) * (n_params + len(out_names))
    out_specs = (bass2jax.PartitionSpec("core"),) * len(out_names)
    donate = tuple(range(n_params, n_params + len(out_names)))

    from jax.sharding import NamedSharding, PartitionSpec as P
    shc = NamedSharding(mesh, P("core"))
    arg_specs = [
        jax.ShapeDtypeStruct((N_CORES * s[0],) + tuple(s[1:]), d, sharding=shc)
        for s, d in in_shapes + out_shapes]

    def compile_fn():
        jfn = jax.jit(
            bass2jax.shard_map(_body, mesh=mesh, in_specs=in_specs,
                               out_specs=out_specs, check_rep=False),
            donate_argnums=donate, keep_unused=True)
        return jfn.lower(*arg_specs).compile()

    try:
        fn = bass2jax.fast_dispatch_compile(compile_fn)
    except Exception:
        fn = jax.jit(
            bass2jax.shard_map(_body, mesh=mesh, in_specs=in_specs,
                               out_specs=out_specs, check_rep=False),
            donate_argnums=donate, keep_unused=True)
    runner = (fn, in_names, out_names, out_shapes, n_params)
    _RUNNER_CACHE[NT] = runner
    return runner


def _get_gather_fn(NT):
    if NT in _GATHER_CACHE:
        return _GATHER_CACHE[NT]
    jax = _init_jax()
    from concourse import bass2jax
    P = bass2jax.PartitionSpec

    def body(tbl, ix):
        g = tbl[ix[:, 0, :]]                      # [128, NT, 64]
        net = g.reshape(128, NT // 2, 2, D).transpose(2, 3, 1, 0)
        net = net.reshape(128, NT // 2, 128)
        nbr = tbl[ix[:, 1, :]]                    # [128, NT, 64]
        return net, nbr

    fn = jax.jit(bass2jax.shard_map(
        body, mesh=_get_mesh(), in_specs=(P(), P("core")),
        out_specs=(P("core"), P("core")), check_rep=False))
    _GATHER_CACHE[NT] = fn
    return fn


def _run_fast(nc, NT, table_dev, relz_dev, ne, nb):
    """Async chain: h2d indices -> gather jit -> bass jit -> d2h scores."""
    jax = _init_jax()
    from jax.sharding import NamedSharding, PartitionSpec as P
    fn, in_names, out_names, out_shapes, n_params = _get_runner(nc, NT)
    assert in_names == ["net", "nbr", "relcatz"] and out_names == ["scores"]
    ix = np.stack([ne, nb], axis=2).reshape(N_CORES * 128, 2, NT)
    ix_dev = jax.device_put(ix, NamedSharding(_get_mesh(), P("core")))
    net_dev, nbr_dev = _get_gather_fn(NT)(table_dev, ix_dev)
    pool = _SCORES_POOL.pop(NT, None)
    if pool is None:
        pool = np.zeros((N_CORES * 128, NT), np.float32)
    (scores_dev,) = fn(net_dev, nbr_dev, relz_dev, pool)
    out = np.asarray(scores_dev)
    _SCORES_POOL[NT] = scores_dev
    return out


# ---------------------------------------------------------------------------
# Fallback 2: host-side gather, ship the gathered rows (slow but simple)
# ---------------------------------------------------------------------------

def _run_cached(nc, NT, in_maps):
    fn, in_names, out_names, out_shapes, n_params = _get_runner(nc, NT)
    concat_in = [np.concatenate([m[nm] for m in in_maps], axis=0)
                 for nm in in_names]
    zero_outs = [np.zeros((N_CORES * shape[0],) + tuple(shape[1:]), dtype)
                 for shape, dtype in out_shapes]
    outs = fn(*concat_in, *zero_outs)
    results = []
    split = {nm: np.split(np.asarray(outs[i]), N_CORES, axis=0)
             for i, nm in enumerate(out_names)}
    for k in range(N_CORES):
        results.append({nm: split[nm][k] for nm in out_names})
    return results


def _kernel_hostgather(node_table, relation_table, ne, nb, NT):
    if NT not in _PROGRAM_CACHE:
        _PROGRAM_CACHE[NT] = build_program(NT)
    nc = _PROGRAM_CACHE[NT]
    relz = _build_relcatz(relation_table)
    in_maps = []
    for k in range(N_CORES):
        rows = node_table[ne[k]]                       # [128(b), NT, 64]
        r4 = rows.reshape(128, NT // 2, 2, D)          # [b, q, par, d]
        net = np.ascontiguousarray(
            r4.transpose(2, 3, 1, 0).reshape(128, NT // 2, 128))
        in_maps.append({"net": net, "nbr": node_table[nb[k]], "relcatz": relz})
    try:
        res = _run_cached(nc, NT, in_maps)
    except Exception:
        res = run_bass_kernel_spmd(nc, in_maps, list(range(N_CORES))).results
    return np.concatenate([r["scores"] for r in res], axis=0)


def kernel(node_idx, relation_idx, node_neighbor_idx, node_table, relation_table):
    node_table = np.ascontiguousarray(np.asarray(node_table, np.float32))
    ne, nb, posmap, NT = _prep_host(node_idx, relation_idx, node_neighbor_idx)

    scores = None
    try:
        table_dev = _dev_table(node_table)
        relz_dev = _dev_relz(relation_table)
        scores = _run_fused(NT, table_dev, relz_dev, ne, nb)
    except Exception:
        try:
            if NT not in _PROGRAM_CACHE:
                _PROGRAM_CACHE[NT] = build_program(NT)
            table_dev = _dev_table(node_table)
            relz_dev = _dev_relz(relation_table)
            scores = _run_fast(_PROGRAM_CACHE[NT], NT, table_dev, relz_dev, ne, nb)
        except Exception:
            scores = _kernel_hostgather(node_table, relation_table, ne, nb, NT)

    scores = scores.reshape(N_CORES, 128, NT)
    Btot = np.asarray(node_idx).shape[0]
    out = np.zeros((Btot, 1), np.float32)
    for k in range(N_CORES):
        valid = posmap[k] >= 0
        out[posmap[k][valid], 0] = scores[k][valid]
    return out


# revision 9
# speedup vs baseline: 1.0839x; 1.0839x over previous
"""Trainium2 Bass kernel for nn_Discriminator (embedding_lookup).

Computation per batch element b:
    ne = node_table[node_idx[b]]                  # [64]
    R  = relation_table[relation_idx[b]] as [64, 64]
    nb = node_table[node_neighbor_idx[b]]         # [64]
    out[b] = sigmoid( (ne @ R) . nb )

Strategy (8 NeuronCores, data-parallel over the batch):
  * The axon tunnel to the devices has ~50-80ms RTT per synchronous op but
    pipelines async work, so the per-call critical path is engineered to be
    one h2d (0.6MB of int32 indices) -> one fused XLA executable -> one d2h
    (0.29MB of scores), enqueued asynchronously with a single sync (~1 RTT).
  * The 25MB node table is replicated and kept DEVICE-RESIDENT across calls
    (cache keyed by object identity, then crc32 of the bytes) per the
    sharding hint; uploads go up sharded (25MB on the wire) and are
    replicated on-device by an all_gather over the chip interconnect. The
    block-diagonalized relation table is likewise device-resident.
  * The embedding gather + transposed `net` layout and the Bass kernel are
    FUSED into one XLA executable: the Bass program is built with
    target_bir_lowering=True so it embeds as an NKI custom_bir_kernel in
    the jit (a plain bass_exec NEFF costs an extra ~70ms round trip per
    execute on this tunnel; a fused NEFF pipelines like any XLA exec).
    Compiles are cached on disk (~3s reload in a fresh process).
  * Host: stable-sort batch by relation_idx, deal round-robin to 8 cores so
    each core's 8192 elements are relation-sorted; pad each of the 8 relation
    groups to a common capacity C (multiple of 128) -> 8*C slots = NT tiles
    of 128 elements (slot s -> partition s%128, tile s//128).
  * Device per core (raw bass, explicit semaphores):
      - stream net/nb span-chunks in via HWDGE DMAs (sync + scalar engines),
      - PE: one matmul per tile-pair: lhsT = net pair [128(2x64 d), 128(batch)],
        rhs = block-diagonal stacked relations -> temp [128, 128] in PSUM,
      - DVE: multiply+reduce temp x NB over 512-wide PSUM spans,
      - ACT: sigmoid, one DMA out of the [128, NT] score block.
  * Host: inverse-permute scores back to batch order.
"""
import sys, os, zlib

for _p in ("/opt/trn_rl_repo", "/root/.axon_site/_ro/trn_rl_repo"):
    if os.path.isdir(_p) and _p not in sys.path:
        sys.path.insert(0, _p)

import numpy as np
import concourse.bass as bass
import concourse.mybir as mybir
from concourse.bass_utils import run_bass_kernel_spmd

NODE_SIZE = 100000
D = 64
N_REL = 8
B = 65536
N_CORES = 8

_JAX_CACHE_DIR = "/var/tmp/jax_cache"

_PROGRAM_CACHE = {}


def build_program(NT, bir_lowering=False, scores_bf16=False):
    """Per-core program. NT: number of 128-element tiles (multiple of 8).
    scores_bf16 halves the d2h payload (sigmoid output rel err ~4e-3,
    well inside the 2e-2 gate)."""
    assert NT % 8 == 0
    NPAIR = NT // 2
    NSPAN = NT // 8      # 8 tiles (4 pairs, 512 temp columns) per span
    NCH = NSPAN          # one DMA chunk per span
    TPG = NT // N_REL    # tiles per relation group

    f32 = mybir.dt.float32
    sc_dt = mybir.dt.bfloat16 if scores_bf16 else f32

    nc = bass.Bass(target_bir_lowering=bir_lowering)
    # net[c, q, b]: partition c = par*64+d holds NE[d] of tile 2q+par, element b
    net_in = nc.dram_tensor("net", [128, NPAIR, 128], f32, kind="ExternalInput")
    nb_in = nc.dram_tensor("nbr", [128, NT, D], f32, kind="ExternalInput")
    # relcatz[:, g*128+0:64] = [R_g; 0], relcatz[:, g*128+64:128] = [0; R_g]
    relcatz = nc.dram_tensor("relcatz", [128, N_REL * 128], f32, kind="ExternalInput")
    out_sc = nc.dram_tensor("scores", [128, NT], sc_dt, kind="ExternalOutput")

    # per-span matmul-instruction counts (pairs crossing a group boundary
    # need two half-width matmuls)
    def pair_tiles(q):
        return 2 * q, 2 * q + 1

    mm_per_span = [0] * NSPAN
    for q in range(NPAIR):
        tA, tB = pair_tiles(q)
        mm_per_span[tA // 8] += 1 if (tA // TPG == tB // TPG) else 2
    cum_mm = np.cumsum([0] + mm_per_span).tolist()

    from contextlib import ExitStack
    with ExitStack() as stack:
        ec = stack.enter_context
        s_relz = ec(nc.sbuf_tensor("sb_relz", [128, N_REL * 128], f32))
        s_net = ec(nc.sbuf_tensor("sb_net", [128, NPAIR, 128], f32))
        s_nb = ec(nc.sbuf_tensor("sb_nb", [128, NT, D], f32))
        s_prod = ec(nc.sbuf_tensor("sb_prod", [128, 8, D], f32))
        s_ssum = ec(nc.sbuf_tensor("sb_ssum", [128, NT], f32))
        s_scores = ec(nc.sbuf_tensor("sb_scores", [128, NT], sc_dt))
        ps_tm = [ec(nc.psum_tensor(f"ps_tm{i}", [128, 512], f32)) for i in range(4)]
        s_ld = ec(nc.semaphore("s_ld"))
        s_mm = ec(nc.semaphore("s_mm"))
        s_dv = ec(nc.semaphore("s_dv"))
        s_pv = ec(nc.semaphore("s_pv"))
        s_sg = ec(nc.semaphore("s_sg"))
        s_out = ec(nc.semaphore("s_out"))
        block = ec(nc.Block())
        s_gc = [nc.alloc_semaphore(f"s_gc{c}") for c in range(NCH)]

        @block.sync
        def _(sync):
            # relz quartered across both HWDGE queues: shortens the head-of-line
            # delay ahead of the first net/nb chunks (-1.9us in the cost model)
            sync.dma_start(s_relz[:, 0:256], relcatz[:, 0:256]).then_inc(s_ld, 16)
            sync.dma_start(s_relz[:, 256:512], relcatz[:, 256:512]).then_inc(s_ld, 16)
            for c in range(NCH):
                sync.dma_start(
                    s_net[:, 4 * c: 4 * c + 4, :], net_in[:, 4 * c: 4 * c + 4, :]
                ).then_inc(s_gc[c], 16)
            sync.wait_ge(s_sg, NSPAN)
            sync.dma_start(out_sc[:], s_scores[:]).then_inc(s_out, 16)
            sync.wait_ge(s_out, 16)

        @block.scalar
        def _(scalar):
            scalar.dma_start(s_relz[:, 512:768], relcatz[:, 512:768]).then_inc(s_ld, 16)
            scalar.dma_start(s_relz[:, 768:1024], relcatz[:, 768:1024]).then_inc(s_ld, 16)
            for c in range(NCH):
                scalar.dma_start(
                    s_nb[:, 8 * c: 8 * c + 8, :], nb_in[:, 8 * c: 8 * c + 8, :]
                ).then_inc(s_gc[c], 16)
            for sp in range(NSPAN):
                scalar.wait_ge(s_dv, sp + 1)
                nc.scalar.activation(
                    s_scores[:, sp * 8: sp * 8 + 8],
                    s_ssum[:, sp * 8: sp * 8 + 8],
                    mybir.ActivationFunctionType.Sigmoid,
                ).then_inc(s_sg)

        @block.tensor
        def _(tensor):
            tensor.wait_ge(s_ld, 64)
            for sp in range(NSPAN):
                tensor.wait_ge(s_gc[sp], 32)
                if sp >= 4:
                    tensor.wait_ge(s_dv, sp - 3)  # WAR: temp bank reuse
                bank = ps_tm[sp % 4]
                cb = 0
                for q in range(4 * sp, 4 * sp + 4):
                    tA, tB = pair_tiles(q)
                    gA, gB = tA // TPG, tB // TPG
                    lhsT = s_net[:, q, :]
                    if gA == gB:
                        nc.tensor.matmul(
                            out=bank[:, cb + (tA % 8) * 64: cb + (tA % 8) * 64 + 128],
                            lhsT=lhsT,
                            rhs=s_relz[:, gA * 128: gA * 128 + 128],
                            start=True, stop=True,
                        ).then_inc(s_mm)
                    else:
                        nc.tensor.matmul(
                            out=bank[:, cb + (tA % 8) * 64: cb + (tA % 8) * 64 + 64],
                            lhsT=lhsT,
                            rhs=s_relz[:, gA * 128: gA * 128 + 64],
                            start=True, stop=True,
                        ).then_inc(s_mm)
                        nc.tensor.matmul(
                            out=bank[:, cb + (tB % 8) * 64: cb + (tB % 8) * 64 + 64],
                            lhsT=lhsT,
                            rhs=s_relz[:, gB * 128 + 64: gB * 128 + 128],
                            start=True, stop=True,
                        ).then_inc(s_mm)

        @block.vector
        def _(vector):
            for sp in range(NSPAN):
                vector.wait_ge(s_mm, cum_mm[sp + 1])
                vector.wait_ge(s_gc[sp], 32)       # NB chunk loaded
                if sp >= 1:
                    vector.wait_ge(s_dv, sp)       # WAR: prod reuse
                nc.vector.tensor_tensor(
                    out=s_prod[:, :, :],
                    in0=ps_tm[sp % 4][:].rearrange("p (a b) -> p a b", a=8),
                    in1=s_nb[:, sp * 8: sp * 8 + 8, :],
                    op=mybir.AluOpType.mult,
                ).then_inc(s_pv)
                vector.wait_ge(s_pv, sp + 1)
                nc.vector.tensor_reduce(
                    out=s_ssum[:, sp * 8: sp * 8 + 8],
                    in_=s_prod[:, :, :],
                    axis=mybir.AxisListType.X,
                    op=mybir.AluOpType.add,
                ).then_inc(s_dv)

    return nc


def _prep_host(node_idx, relation_idx, node_neighbor_idx):
    """Sort by relation, deal to cores, pad groups. Returns per-core int32
    index arrays [128, NT], posmap [N_CORES, 128, NT] (-1 = padding), NT."""
    node_idx = np.asarray(node_idx).astype(np.int32)
    relation_idx = np.asarray(relation_idx).astype(np.uint8)  # values < 8
    node_neighbor_idx = np.asarray(node_neighbor_idx).astype(np.int32)

    order = np.argsort(relation_idx, kind="stable")  # radix path on uint8
    core_pos = [order[k::N_CORES] for k in range(N_CORES)]
    counts = np.zeros((N_CORES, N_REL), np.int64)
    for k in range(N_CORES):
        counts[k] = np.bincount(relation_idx[core_pos[k]], minlength=N_REL)
    C = max(int(np.ceil(counts.max() / 128.0) * 128), 128)
    NT = (N_REL * C) // 128

    ne = np.zeros((N_CORES, 128, NT), np.int32)
    nb = np.zeros((N_CORES, 128, NT), np.int32)
    posmap = np.full((N_CORES, 128, NT), -1, np.int64)
    for k in range(N_CORES):
        pos = core_pos[k]
        cnt = counts[k]
        starts = np.repeat(np.arange(N_REL) * C, cnt)
        group_base = np.repeat(np.cumsum(cnt) - cnt, cnt)
        within = np.arange(len(pos)) - group_base
        s = starts + within
        t, p = s // 128, s % 128
        ne[k, p, t] = node_idx[pos]
        nb[k, p, t] = node_neighbor_idx[pos]
        posmap[k, p, t] = pos
    return ne, nb, posmap, NT


def _build_relcatz(relation_table):
    rt = np.asarray(relation_table, np.float32).reshape(N_REL, D, D)
    relz = np.zeros((128, N_REL * 128), np.float32)
    for g in range(N_REL):
        relz[0:64, g * 128: g * 128 + 64] = rt[g]
        relz[64:128, g * 128 + 64: g * 128 + 128] = rt[g]
    return relz


# ---------------------------------------------------------------------------
# jax plumbing: mesh, persistent compile cache, device-resident tables
# ---------------------------------------------------------------------------

_MESH = None
_TABLE_CACHE = {}    # "node"/"relz" -> (crc_key, device_array, host_ref)
_REPL_FN = {}        # n_rows -> jitted all_gather replicator
_FUSED_CACHE = {}    # NT -> fused fast-dispatch Compiled
_RUNNER_CACHE = {}   # NT -> non-lowered bass runner (fallback)
_GATHER_CACHE = {}   # NT -> gather-only jit (fallback)
_SCORES_POOL = {}    # NT -> device array donated as next out-buffer (fallback)


def _init_jax():
    import jax
    try:
        if not jax.config.jax_compilation_cache_dir:
            jax.config.update("jax_compilation_cache_dir", _JAX_CACHE_DIR)
            jax.config.update("jax_persistent_cache_min_compile_time_secs", 0.0)
            jax.config.update("jax_persistent_cache_min_entry_size_bytes", 0)
    except Exception:
        pass
    return jax


def _get_mesh():
    global _MESH
    if _MESH is None:
        jax = _init_jax()
        from concourse import bass2jax
        _MESH = bass2jax.Mesh(np.asarray(jax.devices()[:N_CORES]), ("core",))
    return _MESH


def _crc(a):
    a = np.ascontiguousarray(a)
    return (a.shape, a.dtype.str, zlib.crc32(memoryview(a.reshape(-1))))


def _replicate(arr):
    """Device-replicated copy of a host array: upload sharded (1x bytes on
    the slow tunnel) and all_gather on-device; plain device_put fallback."""
    jax = _init_jax()
    from jax.sharding import NamedSharding, PartitionSpec as P
    mesh = _get_mesh()
    n = arr.shape[0]
    if n % N_CORES == 0:
        try:
            if n not in _REPL_FN:
                from jax.experimental.shard_map import shard_map

                def body(shard):
                    return jax.lax.all_gather(shard, "core", axis=0, tiled=True)

                _REPL_FN[n] = jax.jit(shard_map(
                    body, mesh=mesh, in_specs=(P("core"),), out_specs=P(),
                    check_rep=False))
            shard = jax.device_put(arr, NamedSharding(mesh, P("core")))
            dev = _REPL_FN[n](shard)
            jax.block_until_ready(dev)
            return dev
        except Exception:
            pass
    dev = jax.device_put(arr, NamedSharding(mesh, P()))
    jax.block_until_ready(dev)
    return dev


def _dev_table(node_table):
    """Replicated device-resident node table, cached by identity then crc."""
    hit = _TABLE_CACHE.get("node")
    if hit is not None and hit[2] is node_table:
        return hit[1]
    key = _crc(node_table)
    if hit is not None and hit[0] == key:
        _TABLE_CACHE["node"] = (key, hit[1], node_table)
        return hit[1]
    dev = _replicate(node_table)
    _TABLE_CACHE["node"] = (key, dev, node_table)
    return dev


def _dev_relz(relation_table):
    """P('core')-sharded [8*128, 1024] block-diagonal relation table."""
    jax = _init_jax()
    from jax.sharding import NamedSharding, PartitionSpec as P
    hit = _TABLE_CACHE.get("relz")
    if hit is not None and hit[2] is relation_table:
        return hit[1]
    key = _crc(np.asarray(relation_table, np.float32))
    if hit is not None and hit[0] == key:
        _TABLE_CACHE["relz"] = (key, hit[1], relation_table)
        return hit[1]
    relz = _build_relcatz(relation_table)
    tiled = np.tile(relz, (N_CORES, 1))
    dev = jax.device_put(tiled, NamedSharding(_get_mesh(), P("core")))
    jax.block_until_ready(dev)
    _TABLE_CACHE["relz"] = (key, dev, relation_table)
    return dev


# ---------------------------------------------------------------------------
# Primary path: gather + bass kernel fused into ONE XLA executable via the
# NKI/BIR-lowering pipeline, AOT-compiled with the bass effect suppressed.
# ---------------------------------------------------------------------------

def _get_fused(NT):
    if NT in _FUSED_CACHE:
        return _FUSED_CACHE[NT]
    jax = _init_jax()
    from concourse import bass2jax
    from jax.sharding import NamedSharding, PartitionSpec as P
    bass2jax.install_neuronx_cc_hook()
    mesh = _get_mesh()
    shc = NamedSharding(mesh, P("core"))

    nc = build_program(NT, bir_lowering=True, scores_bf16=True)
    if not nc.is_finalized():
        nc.finalize()
    in_names, out_names, out_avals = [], [], []
    pn = nc.partition_id_tensor.name if nc.partition_id_tensor else None
    for alloc in nc.m.functions[0].allocations:
        if not isinstance(alloc, mybir.MemoryLocationSet):
            continue
        name = alloc.memorylocations[0].name
        if alloc.kind == "ExternalInput":
            if name != pn:
                in_names.append(name)
        elif alloc.kind == "ExternalOutput":
            out_names.append(name)
            out_avals.append(jax.core.ShapedArray(
                tuple(alloc.tensor_shape), mybir.dt.np(alloc.dtype)))
    assert in_names == ["net", "nbr", "relcatz"] and out_names == ["scores"]
    all_names = list(in_names) + ([pn] if pn else [])

    def fused_body(tbl, ix, relz):
        g = tbl[ix[:, 0, :]]                       # [128, NT, 64]
        net = g.reshape(128, NT // 2, 2, D).transpose(2, 3, 1, 0)
        net = net.reshape(128, NT // 2, 128)
        nbr = tbl[ix[:, 1, :]]                     # [128, NT, 64]
        operands = [net, nbr, relz]
        if pn:
            operands.append(bass2jax.partition_id_tensor())
        outs = bass2jax._bass_exec_p.bind(
            *operands, out_avals=tuple(out_avals), in_names=tuple(all_names),
            out_names=tuple(out_names), lowering_input_output_aliases=(),
            sim_require_finite=True, sim_require_nnan=True, nc=nc)
        return outs[0]

    arg_specs = [
        jax.ShapeDtypeStruct((NODE_SIZE, D), np.float32,
                             sharding=NamedSharding(mesh, P())),
        jax.ShapeDtypeStruct((N_CORES * 128, 2, NT), np.int32, sharding=shc),
        jax.ShapeDtypeStruct((N_CORES * 128, N_REL * 128), np.float32,
                             sharding=shc),
    ]

    def compile_fn():
        jfn = jax.jit(bass2jax.shard_map(
            fused_body, mesh=mesh, in_specs=(P(), P("core"), P("core")),
            out_specs=P("core"), check_rep=False))
        return jfn.lower(*arg_specs).compile()

    fn = bass2jax.fast_dispatch_compile(compile_fn)
    _FUSED_CACHE[NT] = fn
    return fn


def _run_fused(NT, table_dev, relz_dev, ne, nb):
    jax = _init_jax()
    from jax.sharding import NamedSharding, PartitionSpec as P
    fn = _get_fused(NT)
    ix = np.stack([ne, nb], axis=2).reshape(N_CORES * 128, 2, NT)
    ix_dev = jax.device_put(ix, NamedSharding(_get_mesh(), P("core")))
    return np.asarray(fn(table_dev, ix_dev, relz_dev)).astype(np.float32)


# ---------------------------------------------------------------------------
# Fallback 1: separate gather jit + non-lowered bass NEFF (device-resident)
# ---------------------------------------------------------------------------

def _get_runner(nc, NT):
    """Cached jitted executor for the non-lowered program."""
    if NT in _RUNNER_CACHE:
        return _RUNNER_CACHE[NT]
    jax = _init_jax()
    from concourse import bass2jax
    bass2jax.install_neuronx_cc_hook()
    in_names, out_names, out_avals, out_shapes, in_shapes = [], [], [], [], []
    partition_name = nc.partition_id_tensor.name if nc.partition_id_tensor else None
    for alloc in nc.m.functions[0].allocations:
        if not isinstance(alloc, mybir.MemoryLocationSet):
            continue
        name = alloc.memorylocations[0].name
        if alloc.kind == "ExternalInput":
            if name != partition_name:
                in_names.append(name)
                in_shapes.append((tuple(alloc.tensor_shape), mybir.dt.np(alloc.dtype)))
        elif alloc.kind == "ExternalOutput":
            shape = tuple(alloc.tensor_shape)
            dtype = mybir.dt.np(alloc.dtype)
            out_names.append(name)
            out_avals.append(jax.core.ShapedArray(shape, dtype))
            out_shapes.append((shape, dtype))
    n_params = len(in_names)
    all_names = list(in_names) + list(out_names)
    if partition_name is not None:
        all_names.append(partition_name)

    def _body(*args):
        operands = list(args)
        if partition_name is not None:
            operands.append(bass2jax.partition_id_tensor())
        outs = bass2jax._bass_exec_p.bind(
            *operands, out_avals=tuple(out_avals), in_names=tuple(all_names),
            out_names=tuple(out_names), lowering_input_output_aliases=(),
            sim_require_finite=True, sim_require_nnan=True, nc=nc)
        return tuple(outs)

    mesh = _get_mesh()
    in_specs = (bass2jax.PartitionSpec("core"),) * (n_params + len(out_names))
    out_specs = (bass2jax.PartitionSpec("core"),) * len(out_names)
    donate = tuple(range(n_params, n_params + len(out_names)))

    from jax.sharding import NamedSharding, PartitionSpec as P
    shc = NamedSharding(mesh, P("core"))
    arg_specs = [
        jax.ShapeDtypeStruct((N_CORES * s[0],) + tuple(s[1:]), d, sharding=shc)
        for s, d in in_shapes + out_shapes]

    def compile_fn():
        jfn = jax.jit(
            bass2jax.shard_map(_body, mesh=mesh, in_specs=in_specs,
                               out_specs=out_specs, check_rep=False),
            donate_argnums=donate, keep_unused=True)
        return jfn.lower(*arg_specs).compile()

    try:
        fn = bass2jax.fast_dispatch_compile(compile_fn)
    except Exception:
        fn = jax.jit(
            bass2jax.shard_map(_body, mesh=mesh, in_specs=in_specs,
                               out_specs=out_specs, check_rep=False),
            donate_argnums=donate, keep_unused=True)
    runner = (fn, in_names, out_names, out_shapes, n_params)
    _RUNNER_CACHE[NT] = runner
    return runner


def _get_gather_fn(NT):
    if NT in _GATHER_CACHE:
        return _GATHER_CACHE[NT]
    jax = _init_jax()
    from concourse import bass2jax
    P = bass2jax.PartitionSpec

    def body(tbl, ix):
        g = tbl[ix[:, 0, :]]                      # [128, NT, 64]
        net = g.reshape(128, NT // 2, 2, D).transpose(2, 3, 1, 0)
        net = net.reshape(128, NT // 2, 128)
        nbr = tbl[ix[:, 1, :]]                    # [128, NT, 64]
        return net, nbr

    fn = jax.jit(bass2jax.shard_map(
        body, mesh=_get_mesh(), in_specs=(P(), P("core")),
        out_specs=(P("core"), P("core")), check_rep=False))
    _GATHER_CACHE[NT] = fn
    return fn


def _run_fast(nc, NT, table_dev, relz_dev, ne, nb):
    """Async chain: h2d indices -> gather jit -> bass jit -> d2h scores."""
    jax = _init_jax()
    from jax.sharding import NamedSharding, PartitionSpec as P
    fn, in_names, out_names, out_shapes, n_params = _get_runner(nc, NT)
    assert in_names == ["net", "nbr", "relcatz"] and out_names == ["scores"]
    ix = np.stack([ne, nb], axis=2).reshape(N_CORES * 128, 2, NT)
    ix_dev = jax.device_put(ix, NamedSharding(_get_mesh(), P("core")))
    net_dev, nbr_dev = _get_gather_fn(NT)(table_dev, ix_dev)
    pool = _SCORES_POOL.pop(NT, None)
    if pool is None:
        pool = np.zeros((N_CORES * 128, NT), np.float32)
    (scores_dev,) = fn(net_dev, nbr_dev, relz_dev, pool)
    out = np.asarray(scores_dev)
    _SCORES_POOL[NT] = scores_dev
    return out


# ---------------------------------------------------------------------------
# Fallback 2: host-side gather, ship the gathered rows (slow but simple)
# ---------------------------------------------------------------------------

def _run_cached(nc, NT, in_maps):
    fn, in_names, out_names, out_shapes, n_params = _get_runner(nc, NT)
    concat_in = [np.concatenate([m[nm] for m in in_maps], axis=0)
                 for nm in in_names]
    zero_outs = [np.zeros((N_CORES * shape[0],) + tuple(shape[1:]), dtype)
                 for shape, dtype in out_shapes]
    outs = fn(*concat_in, *zero_outs)
    results = []
    split = {nm: np.split(np.asarray(outs[i]), N_CORES, axis=0)
             for i, nm in enumerate(out_names)}
    for k in range(N_CORES):
        results.append({nm: split[nm][k] for nm in out_names})
    return results


def _kernel_hostgather(node_table, relation_table, ne, nb, NT):
    if NT not in _PROGRAM_CACHE:
        _PROGRAM_CACHE[NT] = build_program(NT)
    nc = _PROGRAM_CACHE[NT]
    relz = _build_relcatz(relation_table)
    in_maps = []
    for k in range(N_CORES):
        rows = node_table[ne[k]]                       # [128(b), NT, 64]
        r4 = rows.reshape(128, NT // 2, 2, D)          # [b, q, par, d]
        net = np.ascontiguousarray(
            r4.transpose(2, 3, 1, 0).reshape(128, NT // 2, 128))
        in_maps.append({"net": net, "nbr": node_table[nb[k]], "relcatz": relz})
    try:
        res = _run_cached(nc, NT, in_maps)
    except Exception:
        res = run_bass_kernel_spmd(nc, in_maps, list(range(N_CORES))).results
    return np.concatenate([r["scores"] for r in res], axis=0)


def kernel(node_idx, relation_idx, node_neighbor_idx, node_table, relation_table):
    node_table = np.ascontiguousarray(np.asarray(node_table, np.float32))
    ne, nb, posmap, NT = _prep_host(node_idx, relation_idx, node_neighbor_idx)

    scores = None
    try:
        table_dev = _dev_table(node_table)
        relz_dev = _dev_relz(relation_table)
        scores = _run_fused(NT, table_dev, relz_dev, ne, nb)
    except Exception:
        try:
            if NT not in _PROGRAM_CACHE:
                _PROGRAM_CACHE[NT] = build_program(NT)
            table_dev = _dev_table(node_table)
            relz_dev = _dev_relz(relation_table)
            scores = _run_fast(_PROGRAM_CACHE[NT], NT, table_dev, relz_dev, ne, nb)
        except Exception:
            scores = _kernel_hostgather(node_table, relation_table, ne, nb, NT)

    scores = scores.reshape(N_CORES, 128, NT)
    Btot = np.asarray(node_idx).shape[0]
    out = np.zeros((Btot, 1), np.float32)
    for k in range(N_CORES):
        valid = posmap[k] >= 0
        out[posmap[k][valid], 0] = scores[k][valid]
    return out
